# revision 23
# baseline (speedup 1.0000x reference)
"""Trainium2 Bass kernel for the DSSM (dual-modality Mamba-style 2D selective
scan) module. 8-core SPMD: scan channels d-sharded (24/core x 4 directions),
upstream in_proj/dwconv d-sharded with modalities packed into partitions,
downstream LN/out position-sharded. Cross-core: 3 column-sliced AllReduces of
x_dbl partials (pipelined against phase A) and one AllToAll (y reshard).
All wide matmuls run as float32r (full-rate fp32 on the PE array).
"""
import sys
sys.path.insert(0, "/opt/trn_rl_repo")
import numpy as np
import concourse.bass as bass
from concourse import mybir
from concourse.bacc import Bacc
from concourse.tile import TileContext
from concourse.bass_utils import run_bass_kernel_spmd

F32 = mybir.dt.float32
R32 = mybir.dt.float32r
F16 = mybir.dt.float16
AF = mybir.ActivationFunctionType
OP = mybir.AluOpType

NCORES = 8
RG = [list(range(NCORES))]
B, H, W = 1, 48, 48
HW = H * W                      # 2304
L = 2 * HW                      # 4608
DM = 96                         # d_model
DI = 192                        # d_inner
NST = 4                         # d_state
RNK = 6                         # dt_rank
K = 4
DSL = DI // NCORES              # 24 channels per core
LANES = NST * DSL               # 96 scan lanes (lane = n*DSL + d)
CH = 512                        # phase-B column chunk
NCH = L // CH                   # 9
PC = HW // NCORES               # 288 positions per core (phase C)
RCH = 480                       # phase-A chunk = 10 image rows
ROWCHUNKS = [(0, 10), (10, 10), (20, 10), (30, 10), (40, 8)]
# r1 layout: [84 rows, spatial cols] split in 3 col slices (pipelined AR).
# rows: 6 groups x 14 (dts 0:6 | B 6:10 | C 10:14), group order:
# (k0,sub) (k0,vi) (k1,sub) (k1,ir) (k2,vi) (k3,ir)
GR = 84
SLC = 960                       # r1 col-slice width (2 rowchunks)
SL_COLS = [960, 960, 388]       # slice2: 384 spatial + 4 attn cols
ROWBASE = {(0, 0): 0, (0, 1): 14, (1, 0): 28, (1, 1): 42,
           (2, 0): 56, (2, 1): 70}  # (tile, half) -> r1 row group base
MODOFF = {"sub": 0, "vi": 32, "ir": 64}  # 32-aligned partition blocks

_cache = {}


def _patch_act_tables():
    import concourse.bacc as _bacc
    from concourse.hw_specs import get_activation_tables as _gat
    if getattr(_bacc, "_act_tables_patched", False):
        return
    def patched(arch):
        tabs = {k: set(v) for k, v in _gat(arch).items()}
        # Force exp/ln to resolve to the combined natural_log_exp table so
        # softplus chains (exp -> ln -> exp) never reload act tables.
        for name in ("exp_and_others", "exp_and_friends"):
            if name in tabs:
                tabs[name].discard(AF.Exp)
        if "natural_log" in tabs:
            tabs["natural_log"].discard(AF.Ln)
        return tabs
    _bacc.get_activation_tables = patched
    _bacc._act_tables_patched = True


def _build():
    _patch_act_tables()
    nc = Bacc(trn_type="TRN2", num_devices=NCORES)
    EIn = dict(kind="ExternalInput")
    # per-core inputs (host-prepped)
    i_xvt = nc.dram_tensor("xvt", [DM, HW], F16, **EIn)
    i_xit = nc.dram_tensor("xit", [DM, HW], F16, **EIn)
    i_wxA = nc.dram_tensor("wxA", [DM, 96], F16, **EIn)  # x in_proj vs xvt
    i_wxB = nc.dram_tensor("wxB", [DM, 96], F16, **EIn)  # x in_proj vs xit
    i_wzA = nc.dram_tensor("wzA", [DM, 48], F16, **EIn)  # z in_proj vs xvt
    i_wzB = nc.dram_tensor("wzB", [DM, 48], F16, **EIn)  # z in_proj vs xit
    i_w9p = nc.dram_tensor("w9p", [96, 9, 96], F16, **EIn)  # conv diag, mods packed
    i_b72p = nc.dram_tensor("b72p", [96, 1], F32, **EIn)
    i_wpk84 = nc.dram_tensor("wpk84", [96, GR], F32, **EIn)  # x_dbl block lhsT
    i_wdtr = nc.dram_tensor("wdtr", [RNK, K, LANES], F16, **EIn)
    i_dtb = nc.dram_tensor("dtb", [LANES, K], F32, **EIn)
    i_asc = nc.dram_tensor("asc", [LANES, K], F32, **EIn)
    i_rep24 = nc.dram_tensor("rep24", [DSL, LANES], F16, **EIn)
    i_repb = nc.dram_tensor("repb", [NST, LANES], F16, **EIn)
    i_m96 = nc.dram_tensor("m96", [LANES, DSL], F16, **EIn)
    i_dvec = nc.dram_tensor("dvec", [DSL, 2], F32, **EIn)  # (vi,ir) summed D
    i_f1q = nc.dram_tensor("f1q", [48, 2, 12], F32, **EIn)  # attn mlp1 per mod
    i_f2 = nc.dram_tensor("f2", [12, 2, 2, DM], F32, **EIn)  # (mod, chunk, out96)
    i_lnw = nc.dram_tensor("lnw", [DM, 2, 4], F32, **EIn)    # (chunk, gvi bvi gir bir)
    i_wout = nc.dram_tensor("wout", [DM, 2, DM], F32, **EIn)  # (contract chunk, out)
    i_wz = nc.dram_tensor("wz", [DM, 4, DM], F16, **EIn)     # z lhsT (vi0,vi1,ir0,ir1)
    i_onec = nc.dram_tensor("onec", [DM, 1], F16, **EIn)
    i_oner = nc.dram_tensor("oner", [1, DM], F32, **EIn)
    i_xvc = nc.dram_tensor("xvc", [DM, PC], F16, **EIn)
    i_xic = nc.dram_tensor("xic", [DM, PC], F16, **EIn)
    o_out = nc.dram_tensor("out", [DM, PC], F32, kind="ExternalOutput")
    # collective DRAM buffers (one pair per r1 col-slice)
    d_r1i = [nc.dram_tensor(f"d_r1i{j}", [GR, SL_COLS[j]], F16)
             for j in range(3)]
    d_r1o = [nc.dram_tensor(f"d_r1o{j}", [GR, SL_COLS[j]], F16,
                            addr_space="Shared") for j in range(3)]
    d_a2iv = nc.dram_tensor("d_a2iv", [NCORES, DSL, PC], F16)
    d_a2ov = nc.dram_tensor("d_a2ov", [NCORES, DSL, PC], F16)
    d_a2ir = nc.dram_tensor("d_a2ir", [NCORES, DSL, PC], F16)
    d_a2or = nc.dram_tensor("d_a2or", [NCORES, DSL, PC], F16)

    import contextlib
    with TileContext(nc) as tc, contextlib.ExitStack() as ctx:
        wpool = ctx.enter_context(tc.tile_pool(name="weights", bufs=1))
        big = ctx.enter_context(tc.tile_pool(name="big", bufs=1))

        # ---- load weights ----
        def wtile(shape, src, dt=F32, eng=None):
            t = wpool.tile(shape, dt, tag=src.name, name="w_" + src.name)
            (eng or nc.sync).dma_start(out=t, in_=src[:].bitcast(dt)
                                       if dt is R32 else src[:])
            return t
        # first: the tensors needed to start in_proj rowchunk 0
        t_wxA = wtile([DM, 96], i_wxA, F16)
        t_wxB = wtile([DM, 96], i_wxB, F16)
        t_wzA = wtile([DM, 48], i_wzA, F16)
        t_wzB = wtile([DM, 48], i_wzB, F16)
        t_xvt = big.tile([DM, HW], F16)
        t_xit = big.tile([DM, HW], F16)
        for (r0, nr) in ROWCHUNKS:
            cs = slice(r0 * W, (r0 + nr) * W)
            nc.sync.dma_start(out=t_xvt[:, cs], in_=i_xvt[:, cs])
            nc.sync.dma_start(out=t_xit[:, cs], in_=i_xit[:, cs])
        t_w9p = wtile([96, 9, 96], i_w9p, F16, nc.gpsimd)
        t_b72p = wtile([96, 1], i_b72p, F32, nc.gpsimd)
        t_wpk84 = wtile([96, GR], i_wpk84, R32, nc.gpsimd)
        t_wdtr = wtile([RNK, K, LANES], i_wdtr, F16, nc.gpsimd)
        t_dtb = wtile([LANES, K], i_dtb, F32, nc.gpsimd)
        t_asc = wtile([LANES, K], i_asc, F32, nc.gpsimd)
        t_rep24 = wtile([DSL, LANES], i_rep24, F16, nc.gpsimd)
        t_repb = wtile([NST, LANES], i_repb, F16, nc.gpsimd)
        t_m96 = wtile([LANES, DSL], i_m96, F16, nc.gpsimd)
        t_dvec = wtile([DSL, 2], i_dvec, F32, nc.gpsimd)
        t_f1q = wtile([48, 2, 12], i_f1q, F32, nc.gpsimd)
        t_f2 = wtile([12, 2, 2, DM], i_f2, F32, nc.gpsimd)
        t_lnw = wtile([DM, 2, 4], i_lnw, F32, nc.gpsimd)
        t_wout = wtile([DM, 2, DM], i_wout, R32, nc.gpsimd)
        t_wz = wtile([DM, 4, DM], i_wz, F16, nc.gpsimd)
        t_onec = wtile([DM, 1], i_onec, F16, nc.gpsimd)
        t_oner = wtile([1, DM], i_oner, R32, nc.gpsimd)
        t_xvc = wtile([DM, PC], i_xvc, F16, nc.gpsimd)
        t_xic = wtile([DM, PC], i_xic, F16, nc.gpsimd)

        # persistent SBUF
        t_xs96 = big.tile([96, HW], R32, tag="xs96")  # rows: sub@0|vi@32|ir@64
        t_xs24 = big.tile([DSL, HW], F16, tag="xs24")  # base-0 f16 copies
        t_xv24 = big.tile([DSL, HW], F16, tag="xv24")
        t_xi24 = big.tile([DSL, HW], F16, tag="xi24")
        t_yvi = big.tile([DSL, HW], F16, tag="yvi")
        t_yir = big.tile([DSL, HW], F16, tag="yir")

        # =========== PHASE A: upstream (d-sharded, mods packed) ===========
        pa1 = ctx.enter_context(tc.tile_pool(name="pa1", bufs=1))
        with tc.tile_pool(name="pain", bufs=2, space="PSUM") as pain, \
             tc.tile_pool(name="painz", bufs=2, space="PSUM") as painz, \
             tc.tile_pool(name="pacv", bufs=2, space="PSUM") as pacv, \
             tc.tile_pool(name="padb", bufs=2, space="PSUM") as padb:
            t_pad = pa1.tile([96, 50, 50], F16, tag="pad")
            nc.vector.memset(t_pad[:], 0.0)

            t_zc = pa1.tile([48, HW], F32, tag="zc")  # silu(z), vi|ir packed
            t_zacc = pa1.tile([48, len(ROWCHUNKS)], F32, tag="zacc")
            # in_proj (+z) packed: x=[sub|vi|ir], z=[zv|zi]
            for ic, (r0, nr) in enumerate(ROWCHUNKS):
                cols = slice(r0 * W, (r0 + nr) * W)
                p_x = pain.tile([96, RCH], F32, tag="pin")
                nc.tensor.matmul(p_x[:, :nr * W], t_wxA[:], t_xvt[:, cols],
                                 start=True, stop=False)
                nc.tensor.matmul(p_x[:, :nr * W], t_wxB[:], t_xit[:, cols],
                                 start=False, stop=True)
                p_z = painz.tile([48, RCH], F32, tag="pz")
                nc.tensor.matmul(p_z[:, :nr * W], t_wzA[:], t_xvt[:, cols],
                                 start=True, stop=False)
                nc.tensor.matmul(p_z[:, :nr * W], t_wzB[:], t_xit[:, cols],
                                 start=False, stop=True)
                nc.scalar.activation(t_zc[:, cols], p_z[:, :nr * W],
                                     AF.Silu, accum_out=t_zacc[:, ic:ic + 1])
                nc.scalar.copy(
                    t_pad[:, 1 + r0:1 + r0 + nr, 1:49],
                    p_x[:, :nr * W].rearrange("p (a b) -> p a b", a=nr))

            # depthwise conv 3x3 (9 block-diag matmuls) + bias + silu -> xs
            for (r0, nr) in ROWCHUNKS:
                p_c = pacv.tile([96, RCH], F32, tag="pconv")
                for tap in range(9):
                    dy, dx = tap // 3, tap % 3
                    nc.tensor.matmul(
                        p_c[:, :nr * W], t_w9p[:, tap, :],
                        t_pad[:, r0 + dy:r0 + dy + nr, dx:dx + 48],
                        start=(tap == 0), stop=(tap == 8))
                cols = slice(r0 * W, (r0 + nr) * W)
                nc.scalar.activation(
                    t_xs96[:, cols], p_c[:, :nr * W],
                    AF.Silu, bias=t_b72p[:], scale=1.0)
                nc.vector.tensor_copy(t_xs24[:, cols], t_xs96[0:24, cols])
                nc.vector.tensor_copy(t_xv24[:, cols], t_xs96[32:56, cols])
                nc.vector.tensor_copy(t_xi24[:, cols], t_xs96[64:88, cols])

            # x_dbl all 6 groups in one matmul per rowchunk -> r1 slices
            for ic, (r0, nr) in enumerate(ROWCHUNKS):
                p_d = padb.tile([GR, RCH], F32, tag="pxdbl")
                nc.tensor.matmul(p_d[:, :nr * W], t_wpk84[:],
                                 t_xs96[:, r0 * W:(r0 + nr) * W],
                                 start=True, stop=True)
                t_xe = pa1.tile([GR, RCH], F16, tag=f"txe{ic}", name=f"txe{ic}")
                nc.scalar.copy(t_xe[:, :nr * W], p_d[:, :nr * W])
                j = (r0 * W) // SLC
                lc = r0 * W - j * SLC
                nc.sync.dma_start(out=d_r1i[j][:, lc:lc + nr * W],
                                  in_=t_xe[:, :nr * W])
                if ic == 1:
                    nc.gpsimd.collective_compute(
                        "AllReduce", OP.add, RG,
                        ins=[d_r1i[0][:]], outs=[d_r1o[0][:]])
                elif ic == 3:
                    nc.gpsimd.collective_compute(
                        "AllReduce", OP.add, RG,
                        ins=[d_r1i[1][:]], outs=[d_r1o[1][:]])
        # chan-attn pooled stats + v1 partials [12, 4] (own PSUM scope)
        with tc.tile_pool(name="pav1", bufs=1, space="PSUM") as pav1:
            t_pool = pa1.tile([48, 2], F32, tag="tpool")  # (avg, max)
            nc.vector.tensor_reduce(t_pool[:, 0:1], t_zacc[:],
                                    axis=mybir.AxisListType.X, op=OP.add)
            nc.vector.tensor_scalar_mul(t_pool[:, 0:1], t_pool[:, 0:1], 1.0 / HW)
            nc.vector.tensor_reduce(t_pool[:, 1:2], t_zc[:],
                                    axis=mybir.AxisListType.X, op=OP.max)
            t_v1 = pa1.tile([12, 4], F16, tag="tv1")  # (via, vim, ira, irm)
            p_v1 = pav1.tile([12, 4], F32, tag="pv1")
            for mi in range(2):
                for st in range(2):
                    nc.tensor.matmul(p_v1[:, 2 * mi + st:2 * mi + st + 1],
                                     t_f1q[:, mi, :], t_pool[:, st:st + 1],
                                     start=True, stop=True)
            nc.scalar.copy(t_v1[:], p_v1[:])
            nc.sync.dma_start(out=d_r1i[2][0:12, 384:388], in_=t_v1[:])
        nc.gpsimd.collective_compute("AllReduce", OP.add, RG,
                                     ins=[d_r1i[2][:]], outs=[d_r1o[2][:]])

        # z recompute at my positions (independent of scan) — emitted here
        # so it fills engine gaps during the r1 AllReduce stall.
        t_z = {}
        zq = ctx.enter_context(tc.tile_pool(name="zq", bufs=1))
        with tc.tile_pool(name="zp", bufs=2, space="PSUM") as zpp:
            for zi, (mod, ck) in enumerate(
                    (("vi", 0), ("vi", 1), ("ir", 0), ("ir", 1))):
                xt = t_xvc if mod == "vi" else t_xic
                p_z = zpp.tile([DM, PC], F32, tag="pz2")
                nc.tensor.matmul(p_z[:], t_wz[:, zi, :], xt[:],
                                 start=True, stop=True)
                t_e = zq.tile([DM, PC], F32, tag=f"ze{zi}", name=f"ze{zi}")
                nc.scalar.activation(t_e[:], p_z[:], AF.Exp, bias=0.0, scale=-1.0)
                nc.vector.tensor_scalar_add(t_e[:], t_e[:], 1.0)
                t_r = zq.tile([DM, PC], F32, tag=f"zrr{zi}", name=f"zrr{zi}")
                nc.vector.reciprocal(t_r[:], t_e[:])
                tz = zq.tile([DM, PC], F32, tag=f"z{zi}", name=f"z{zi}")
                nc.vector.tensor_mul(tz[:], p_z[:], t_r[:])
                t_z[(mod, ck)] = tz

        # =========== PHASE B: scan middle ===========
        # full-width per-(tile,half) operand tiles; loads ordered by AR slice
        # so the sync queue drains as each sliced AllReduce completes.
        rfp = ctx.enter_context(tc.tile_pool(name="rfull", bufs=1))
        t_Rf, t_Bf, t_Cf = {}, {}, {}
        for tt in range(3):
            for hh in range(2):
                t_Rf[(tt, hh)] = rfp.tile([RNK, HW], F16, tag=f"Rf{tt}{hh}",
                                          name=f"Rf{tt}{hh}")
                t_Bf[(tt, hh)] = rfp.tile([NST, HW], F16, tag=f"Bf{tt}{hh}",
                                          name=f"Bf{tt}{hh}")
                if hh == 1 or tt == 2:  # y-feeding halves need C
                    t_Cf[(tt, hh)] = rfp.tile([NST, HW], F16, tag=f"Cf{tt}{hh}",
                                              name=f"Cf{tt}{hh}")
        for j in range(3):
            g0 = j * SLC
            span = min(SLC, HW - g0)
            for tt in range(3):
                for hh in range(2):
                    rb = ROWBASE[(tt, hh)]
                    nc.sync.dma_start(out=t_Rf[(tt, hh)][:, g0:g0 + span],
                                      in_=d_r1o[j][rb:rb + RNK, 0:span])
                    nc.sync.dma_start(
                        out=t_Bf[(tt, hh)][:, g0:g0 + span],
                        in_=d_r1o[j][rb + RNK:rb + RNK + NST, 0:span])
                    if (tt, hh) in t_Cf:
                        nc.sync.dma_start(
                            out=t_Cf[(tt, hh)][:, g0:g0 + span],
                            in_=d_r1o[j][rb + RNK + NST:rb + 14, 0:span])
        t_v1o = big.tile([12, 4], F16, tag="v1o")
        nc.sync.dma_start(out=t_v1o, in_=d_r1o[2][0:12, 384:388])

        xs_t = {"sub": t_xs24, "vi": t_xv24, "ir": t_xi24}

        def xs_view(t, col, n):
            half = 1 if col >= HW else 0
            mod = (("sub", "vi"), ("sub", "ir"), ("vi", "ir"))[t][half]
            sc = col - HW * half
            return xs_t[mod][0:DSL, sc:sc + n]

        with tc.tile_pool(name="pb", bufs=3) as pb, \
             tc.tile_pool(name="pbd", bufs=2, space="PSUM") as pbd, \
             tc.tile_pool(name="pbp", bufs=1, space="PSUM") as pbp, \
             tc.tile_pool(name="pby", bufs=2, space="PSUM") as pby:
            for t in range(3):
                chunk_order = range(NCH) if t < 2 else range(NCH - 1, -1, -1)
                carry = None
                for c in chunk_order:
                    c0 = c * CH
                    # segment pieces within chunk: (start, end, k) in tile cols
                    k_lo = t if t < 2 else 2
                    k_hi = t if t < 2 else 3
                    if c0 >= HW:
                        pieces = [(c0, c0 + CH, k_hi)]
                    elif c0 + CH <= HW:
                        pieces = [(c0, c0 + CH, k_lo)]
                    else:
                        pieces = [(c0, HW, k_lo), (HW, c0 + CH, k_hi)]

                    p_dts = pbd.tile([LANES, CH], F32, tag="dts")
                    for (s, e, k) in pieces:
                        half = 1 if s >= HW else 0
                        sp0 = s - HW * half
                        nc.tensor.matmul(p_dts[:, s - c0:e - c0], t_wdtr[:, k, :],
                                         t_Rf[(t, half)][:, sp0:sp0 + e - s],
                                         start=True, stop=True)
                    t_et = pb.tile([LANES, CH], F32, tag="et")
                    for (s, e, k) in pieces:
                        nc.scalar.activation(t_et[:, s - c0:e - c0],
                                             p_dts[:, s - c0:e - c0], AF.Exp,
                                             bias=t_dtb[:, k:k + 1], scale=1.0)
                    t_delta = pb.tile([LANES, CH], F32, tag="delta")
                    nc.scalar.activation(t_delta[:], t_et[:], AF.Ln,
                                         bias=1.0, scale=1.0)
                    t_u = pb.tile([DSL, CH], F16, tag="u")
                    for (s, e, _k) in pieces:
                        nc.gpsimd.tensor_mul(t_u[:, s - c0:e - c0],
                                             t_delta[0:DSL, s - c0:e - c0],
                                             xs_view(t, s, e - s))
                    p_u = pbp.tile([LANES, CH], F32, tag="urep")
                    nc.tensor.matmul(p_u[:], t_rep24[:], t_u[:], start=True, stop=True)
                    p_B = pbp.tile([LANES, CH], F32, tag="brep")
                    for (s, e, k) in pieces:
                        half = 1 if s >= HW else 0
                        sp0 = s - HW * half
                        nc.tensor.matmul(p_B[:, s - c0:e - c0], t_repb[:],
                                         t_Bf[(t, half)][:, sp0:sp0 + e - s],
                                         start=True, stop=True)
                    t_bsb = pb.tile([LANES, CH], F16, tag="bsb")
                    nc.scalar.copy(t_bsb[:], p_B[:])
                    t_b = pb.tile([LANES, CH], F16, tag="b")
                    nc.vector.tensor_mul(t_b[:], p_u[:], t_bsb[:])
                    t_a = pb.tile([LANES, CH], F16, tag="a")
                    for (s, e, k) in pieces:
                        nc.scalar.activation(t_a[:, s - c0:e - c0],
                                             t_delta[:, s - c0:e - c0], AF.Exp,
                                             bias=0.0, scale=t_asc[:, k:k + 1])
                    t_h = pb.tile([LANES, CH], F16, tag="h")
                    if t < 2:
                        init = 0.0 if c == 0 else carry[:, CH - 1:CH]
                        nc.vector.tensor_tensor_scan(t_h[:], t_a[:], t_b[:], init,
                                                     OP.mult, OP.add)
                        carry = t_h
                    else:
                        # reverse scan; pieces processed right-to-left
                        for (s, e, k) in reversed(pieces):
                            sl = slice(s - c0, e - c0)
                            if e == L or e == HW:      # scan-time segment start
                                init = 0.0
                            else:
                                init = carry
                            nc.vector.tensor_tensor_scan(
                                t_h[:, sl][:, ::-1], t_a[:, sl][:, ::-1],
                                t_b[:, sl][:, ::-1], init, OP.mult, OP.add)
                            carry = t_h[:, s - c0:s - c0 + 1]

                    # y: only vi/ir halves feed the output
                    ypieces = [((s if t == 2 else max(s, HW)), e, k)
                               for (s, e, k) in pieces if t == 2 or e > HW]
                    if not ypieces:
                        continue
                    y0 = ypieces[0][0] - c0
                    y1 = ypieces[-1][1] - c0
                    p_C = pbp.tile([LANES, CH], F32, tag="crep")
                    for (s, e, k) in ypieces:
                        half = 1 if s >= HW else 0
                        sp0 = s - HW * half
                        nc.tensor.matmul(p_C[:, s - c0:e - c0], t_repb[:],
                                         t_Cf[(t, half)][:, sp0:sp0 + e - s],
                                         start=True, stop=True)
                    t_hc = pb.tile([LANES, CH], F16, tag="hc")
                    nc.vector.tensor_mul(t_hc[:, y0:y1], t_h[:, y0:y1],
                                         p_C[:, y0:y1])
                    p_y = pby.tile([DSL, CH], F32, tag="y")
                    nc.tensor.matmul(p_y[:, y0:y1], t_m96[:], t_hc[:, y0:y1],
                                     start=True, stop=True)
                    # evacuate/accumulate into y_vi / y_ir; on fwd tiles the
                    # D-skip (combined D_k + D_{k+2}) folds into the same op:
                    # y = xs * D + p_y
                    for (s, e, _k) in ypieces:
                        sl = slice(s - c0, e - c0)
                        if t < 2:
                            yt = t_yvi if t == 0 else t_yir
                            nc.vector.scalar_tensor_tensor(
                                yt[:, s - HW:e - HW],
                                xs_view(t, s, e - s),
                                t_dvec[:, t:t + 1], p_y[:, sl],
                                OP.mult, OP.add)
                        elif s < HW:  # t2 k2 -> vi
                            nc.vector.tensor_add(t_yvi[:, s:e], t_yvi[:, s:e],
                                                 p_y[:, sl])
                        else:         # t2 k3 -> ir
                            nc.vector.tensor_add(t_yir[:, s - HW:e - HW],
                                                 t_yir[:, s - HW:e - HW], p_y[:, sl])

        # =========== A2A: reshard y channels -> positions ===========
        for j in range(NCORES):
            nc.sync.dma_start(out=d_a2ir[j, :, :],
                              in_=t_yir[:, j * PC:(j + 1) * PC])
            nc.sync.dma_start(out=d_a2iv[j, :, :],
                              in_=t_yvi[:, j * PC:(j + 1) * PC])
        nc.gpsimd.collective_compute("AllToAll", OP.bypass, RG,
                                     ins=[d_a2ir[:]], outs=[d_a2or[:]])
        nc.gpsimd.collective_compute("AllToAll", OP.bypass, RG,
                                     ins=[d_a2iv[:]], outs=[d_a2ov[:]])

        # =========== PHASE C: LN + gate + out (position-sharded) ===========
        with tc.tile_pool(name="pcq", bufs=2) as pcq, \
             tc.tile_pool(name="pcp", bufs=1, space="PSUM") as pcp:
            # gather y chunks [96, PC] x (2 chunks, 2 mods)
            t_y = {}
            for mod, d_src in (("vi", d_a2ov), ("ir", d_a2or)):
                for ck in range(2):
                    ty = pcq.tile([DM, PC], F16, tag=f"y{mod}{ck}", name=f"y{mod}{ck}")
                    for jj in range(4):
                        j = ck * 4 + jj
                        nc.sync.dma_start(out=ty[jj * DSL:(jj + 1) * DSL, :],
                                          in_=d_src[j, :, :])
                    t_y[(mod, ck)] = ty
            # chan-attn scales s = 1 + sigmoid(f2 @ (relu(va)+relu(vm)))
            t_vr = pcq.tile([12, 4], F32, tag="vr")
            nc.scalar.activation(t_vr[:], t_v1o[:], AF.Relu)
            t_vw = pcq.tile([12, 2], F32, tag="vw")
            nc.vector.tensor_add(t_vw[:, 0:1], t_vr[:, 0:1], t_vr[:, 1:2])
            nc.vector.tensor_add(t_vw[:, 1:2], t_vr[:, 2:3], t_vr[:, 3:4])
            t_s = {}
            for ck in range(2):
                p_ca = pcp.tile([DM, 2], F32, tag="pca")
                for mod_i in range(2):
                    nc.tensor.matmul(p_ca[:, mod_i:mod_i + 1], t_f2[:, mod_i, ck, :],
                                     t_vw[:, mod_i:mod_i + 1],
                                     start=True, stop=True)
                t_e = pcq.tile([DM, 2], F32, tag="cae")
                nc.scalar.activation(t_e[:], p_ca[:], AF.Exp, bias=0.0, scale=-1.0)
                nc.vector.tensor_scalar_add(t_e[:], t_e[:], 1.0)
                t_r = pcq.tile([DM, 2], F32, tag=f"car{ck}", name=f"car{ck}")
                nc.vector.reciprocal(t_r[:], t_e[:])          # sigmoid
                nc.vector.tensor_scalar_add(t_r[:], t_r[:], 1.0)  # 1 + sigmoid
                t_s[ck] = t_r
            # LN per modality
            t_fin = {}
            for mod in ("vi", "ir"):
                p_s1 = pcp.tile([1, PC], F32, tag="s1")
                p_s2 = pcp.tile([1, PC], F32, tag="s2")
                for ck in range(2):
                    nc.tensor.matmul(p_s1[:], t_onec[:],
                                     t_y[(mod, ck)][:], start=(ck == 0),
                                     stop=(ck == 1))
                for ck in range(2):
                    t_sq = pcq.tile([DM, PC], F16, tag="sq")
                    nc.scalar.activation(t_sq[:], t_y[(mod, ck)][:], AF.Square)
                    nc.tensor.matmul(p_s2[:], t_onec[:],
                                     t_sq[:], start=(ck == 0), stop=(ck == 1))
                t_mu = pcq.tile([1, PC], F32, tag="mu")
                nc.vector.tensor_scalar_mul(t_mu[:], p_s1[:], 1.0 / DI)
                t_musq = pcq.tile([1, PC], F32, tag="musq")
                nc.vector.tensor_mul(t_musq[:], t_mu[:], t_mu[:])
                t_var = pcq.tile([1, PC], F32, tag="var")
                nc.vector.scalar_tensor_tensor(t_var[:], p_s2[:], 1.0 / DI,
                                               t_musq[:], OP.mult, OP.subtract)
                t_eps = pcq.tile([1, 1], F32, tag="eps")
                nc.vector.memset(t_eps[:], 1e-5)
                t_lnv = pcq.tile([1, PC], F32, tag="lnv")
                nc.scalar.activation(t_lnv[:], t_var[:], AF.Ln, bias=t_eps[:], scale=1.0)
                t_rstd = pcq.tile([1, PC], R32, tag="rstd")
                nc.scalar.activation(t_rstd[:], t_lnv[:], AF.Exp, bias=0.0, scale=-0.5)
                t_mur = pcq.tile([1, PC], R32, tag="mur")
                nc.vector.tensor_mul(t_mur[:], t_mu[:], t_rstd[:].bitcast(F32))
                p_q = pcp.tile([DM, PC], F32, tag="pq")
                nc.tensor.matmul(p_q[:], t_oner[:], t_rstd[:], start=True, stop=True)
                p_m = pcp.tile([DM, PC], F32, tag="pm")
                nc.tensor.matmul(p_m[:], t_oner[:], t_mur[:], start=True, stop=True)
                gb = {"vi": (0, 1), "ir": (2, 3)}[mod]
                for ck in range(2):
                    t_t = pcq.tile([DM, PC], F32, tag="lt")
                    nc.vector.tensor_mul(t_t[:], t_y[(mod, ck)][:], p_q[:])
                    t_t2 = pcq.tile([DM, PC], F32, tag="lt2")
                    nc.vector.tensor_sub(t_t2[:], t_t[:], p_m[:])
                    t_yn = pcq.tile([DM, PC], F32, tag="yn")
                    nc.scalar.activation(t_yn[:], t_t2[:], AF.Identity,
                                         bias=t_lnw[:, ck, gb[1]:gb[1] + 1],
                                         scale=t_lnw[:, ck, gb[0]:gb[0] + 1])
                    # gate: fin += yn * z * s
                    t_m1 = pcq.tile([DM, PC], F32, tag="m1")
                    nc.vector.tensor_mul(t_m1[:], t_yn[:], t_z[(mod, ck)][:])
                    if mod == "vi":
                        t_f = pcq.tile([DM, PC], F32, tag=f"fin{ck}", name=f"fin{ck}")
                        nc.vector.tensor_scalar_mul(t_f[:], t_m1[:],
                                                    t_s[ck][:, 0:1])
                        t_fin[ck] = t_f
                    else:
                        nc.vector.scalar_tensor_tensor(t_fin[ck][:], t_m1[:],
                                                       t_s[ck][:, 1:2], t_fin[ck][:],
                                                       OP.mult, OP.add)
            p_o = pcp.tile([DM, PC], F32, tag="po")
            for ck in range(2):
                t_finr = pcq.tile([DM, PC], R32, tag=f"finr{ck}", name=f"finr{ck}")
                nc.vector.tensor_copy(t_finr[:], t_fin[ck][:])
                nc.tensor.matmul(p_o[:], t_wout[:, ck, :], t_finr[:],
                                 start=(ck == 0), stop=(ck == 1))
            t_o = pcq.tile([DM, PC], F32, tag="o")
            nc.scalar.copy(t_o[:], p_o[:])
            nc.sync.dma_start(out=o_out[:], in_=t_o[:])

    nc.finalize()
    return nc


def _prep_inputs(inputs):
    """Host-side prep: slice/transpose weights per core. Returns in_maps."""
    g = {k: np.asarray(v, dtype=np.float32) for k, v in inputs.items()}
    x_vi = g["x_vi"].reshape(HW, DM)
    x_ir = g["x_ir"].reshape(HW, DM)
    xvt = np.ascontiguousarray(x_vi.T)
    xit = np.ascontiguousarray(x_ir.T)
    A = -np.exp(g["A_logs"]).reshape(K, DI, NST)
    Ds = g["Ds"].reshape(K, DI)
    in_maps = []
    for c in range(NCORES):
        S = slice(c * DSL, (c + 1) * DSL)
        m = {}
        m["xvt"] = xvt.astype(np.float16)
        m["xit"] = xit.astype(np.float16)
        # packed in_proj lhsT: x blocks sub@0 vi@32 ir@64, z cols [zv | zi]
        wxA = np.zeros((DM, 96), np.float32)
        wxB = np.zeros((DM, 96), np.float32)
        wzA = np.zeros((DM, 48), np.float32)
        wzB = np.zeros((DM, 48), np.float32)
        wxA[:, 0:24] = g["W_sub"][S].T
        wxA[:, 32:56] = g["W_vi"][S].T
        wxB[:, 0:24] = -g["W_sub"][S].T
        wxB[:, 64:88] = g["W_ir"][S].T
        wzA[:, 0:24] = g["W_vi"][DI:][S].T
        wzB[:, 24:48] = g["W_ir"][DI:][S].T
        m["wxA"], m["wxB"] = wxA.astype(np.float16), wxB.astype(np.float16)
        m["wzA"], m["wzB"] = wzA.astype(np.float16), wzB.astype(np.float16)
        w9p = np.zeros((96, 9, 96), np.float32)
        b72p = np.zeros((96, 1), np.float32)
        for nm in ("sub", "vi", "ir"):
            mo = MODOFF[nm]
            cw = g[f"conv_w_{nm}"][S, 0]      # [DSL, 3, 3]
            for tap in range(9):
                for d in range(DSL):
                    w9p[mo + d, tap, mo + d] = cw[d, tap // 3, tap % 3]
            b72p[mo:mo + DSL, 0] = g[f"conv_b_{nm}"][S]
        m["w9p"], m["b72p"] = w9p.astype(np.float16), b72p
        # x_dbl block lhsT: 6 groups (k0s k0v k1s k1i k2v k3i) x 14 rows
        wpk84 = np.zeros((96, GR), np.float32)
        for gi2, (k, nm) in enumerate(
                ((0, "sub"), (0, "vi"), (1, "sub"), (1, "ir"),
                 (2, "vi"), (3, "ir"))):
            mo = MODOFF[nm]
            wpk84[mo:mo + DSL, gi2 * 14:(gi2 + 1) * 14] = \
                g["x_proj_weight"][k][:, S].T
        m["wpk84"] = wpk84
        wdtr = np.zeros((RNK, K, LANES), np.float32)
        dtb = np.zeros((LANES, K), np.float32)
        asc = np.zeros((LANES, K), np.float32)
        for k in range(K):
            for n in range(NST):
                for d in range(DSL):
                    lane = n * DSL + d
                    wdtr[:, k, lane] = g["dt_projs_weight"][k, c * DSL + d, :]
                    dtb[lane, k] = g["dt_projs_bias"][k, c * DSL + d]
                    asc[lane, k] = A[k, c * DSL + d, n]
        m["wdtr"] = wdtr.astype(np.float16)
        m["dtb"], m["asc"] = dtb, asc
        rep24 = np.zeros((DSL, LANES), np.float32)
        repb = np.zeros((NST, LANES), np.float32)
        m96 = np.zeros((LANES, DSL), np.float32)
        for n in range(NST):
            for d in range(DSL):
                rep24[d, n * DSL + d] = 1
                repb[n, n * DSL + d] = 1
                m96[n * DSL + d, d] = 1
        m["rep24"], m["repb"], m["m96"] = (rep24.astype(np.float16),
            repb.astype(np.float16), m96.astype(np.float16))
        dvec = np.zeros((DSL, 2), np.float32)
        dvec[:, 0] = Ds[0, S] + Ds[2, S]
        dvec[:, 1] = Ds[1, S] + Ds[3, S]
        m["dvec"] = dvec
        f1q = np.zeros((48, 2, 12), np.float32)
        f1q[0:24, 0] = g["ca_vi_f1"][:, S].T
        f1q[24:48, 1] = g["ca_ir_f1"][:, S].T
        m["f1q"] = f1q
        f2 = np.zeros((12, 2, 2, DM), np.float32)
        for ck in range(2):
            f2[:, 0, ck] = g["ca_vi_f2"][ck * DM:(ck + 1) * DM].T
            f2[:, 1, ck] = g["ca_ir_f2"][ck * DM:(ck + 1) * DM].T
        m["f2"] = f2
        lnw = np.zeros((DM, 2, 4), np.float32)
        for ck in range(2):
            cs = slice(ck * DM, (ck + 1) * DM)
            lnw[:, ck, 0] = g["ln_vi_g"][cs]
            lnw[:, ck, 1] = g["ln_vi_b"][cs]
            lnw[:, ck, 2] = g["ln_ir_g"][cs]
            lnw[:, ck, 3] = g["ln_ir_b"][cs]
        m["lnw"] = lnw
        wout = np.zeros((DM, 2, DM), np.float32)
        for ck in range(2):
            wout[:, ck] = g["W_out"][:, ck * DM:(ck + 1) * DM].T
        m["wout"] = wout
        wz = np.zeros((DM, 4, DM), np.float32)
        wz[:, 0] = g["W_vi"][DI:][0:DM].T
        wz[:, 1] = g["W_vi"][DI:][DM:DI].T
        wz[:, 2] = g["W_ir"][DI:][0:DM].T
        wz[:, 3] = g["W_ir"][DI:][DM:DI].T
        m["wz"] = wz.astype(np.float16)
        m["onec"] = np.ones((DM, 1), np.float16)
        m["oner"] = np.ones((1, DM), np.float32)
        m["xvc"] = np.ascontiguousarray(xvt[:, c * PC:(c + 1) * PC]).astype(np.float16)
        m["xic"] = np.ascontiguousarray(xit[:, c * PC:(c + 1) * PC]).astype(np.float16)
        in_maps.append(m)
    return in_maps


def kernel(**inputs):
    if "nc" not in _cache:
        _cache["nc"] = _build()
    nc = _cache["nc"]
    in_maps = _prep_inputs(inputs)
    res = run_bass_kernel_spmd(nc, in_maps, core_ids=list(range(NCORES)))
    out = np.zeros((DM, HW), np.float32)
    for c in range(NCORES):
        out[:, c * PC:(c + 1) * PC] = res.results[c]["out"]
    return out.T.reshape(B, H, W, DM).astype(np.float32)


# revision 24
# speedup vs baseline: 1.0252x; 1.0252x over previous
"""Trainium2 Bass kernel for the DSSM (dual-modality Mamba-style 2D selective
scan) module. 8-core SPMD: scan channels d-sharded (24/core x 4 directions),
upstream in_proj/dwconv d-sharded with modalities packed into partitions,
downstream LN/out position-sharded. Cross-core: 3 column-sliced AllReduces of
x_dbl partials (pipelined against phase A) and one AllToAll (y reshard).
All wide matmuls run as float32r (full-rate fp32 on the PE array).
"""
import sys
sys.path.insert(0, "/opt/trn_rl_repo")
import numpy as np
import concourse.bass as bass
from concourse import mybir
from concourse.bacc import Bacc
from concourse.tile import TileContext
from concourse.bass_utils import run_bass_kernel_spmd

F32 = mybir.dt.float32
R32 = mybir.dt.float32r
F16 = mybir.dt.float16
AF = mybir.ActivationFunctionType
OP = mybir.AluOpType

NCORES = 8
RG = [list(range(NCORES))]
B, H, W = 1, 48, 48
HW = H * W                      # 2304
L = 2 * HW                      # 4608
DM = 96                         # d_model
DI = 192                        # d_inner
NST = 4                         # d_state
RNK = 6                         # dt_rank
K = 4
DSL = DI // NCORES              # 24 channels per core
LANES = NST * DSL               # 96 scan lanes (lane = n*DSL + d)
CH = 512                        # phase-B column chunk
NCH = L // CH                   # 9
PC = HW // NCORES               # 288 positions per core (phase C)
RCH = 480                       # phase-A chunk = 10 image rows
ROWCHUNKS = [(0, 10), (10, 10), (20, 10), (30, 10), (40, 8)]
# r1 layout: [84 rows, spatial cols] split in 3 col slices (pipelined AR).
# rows: 6 groups x 14 (dts 0:6 | B 6:10 | C 10:14), group order:
# (k0,sub) (k0,vi) (k1,sub) (k1,ir) (k2,vi) (k3,ir)
GR = 84
SLC = 960                       # r1 col-slice width (2 rowchunks)
SL_COLS = [960, 960, 388]       # slice2: 384 spatial + 4 attn cols
ROWBASE = {(0, 0): 0, (0, 1): 14, (1, 0): 28, (1, 1): 42,
           (2, 0): 56, (2, 1): 70}  # (tile, half) -> r1 row group base
MODOFF = {"sub": 0, "vi": 32, "ir": 64}  # 32-aligned partition blocks

_cache = {}


def _patch_act_tables():
    import concourse.bacc as _bacc
    from concourse.hw_specs import get_activation_tables as _gat
    if getattr(_bacc, "_act_tables_patched", False):
        return
    def patched(arch):
        tabs = {k: set(v) for k, v in _gat(arch).items()}
        # Force exp/ln to resolve to the combined natural_log_exp table so
        # softplus chains (exp -> ln -> exp) never reload act tables.
        for name in ("exp_and_others", "exp_and_friends"):
            if name in tabs:
                tabs[name].discard(AF.Exp)
        if "natural_log" in tabs:
            tabs["natural_log"].discard(AF.Ln)
        return tabs
    _bacc.get_activation_tables = patched
    _bacc._act_tables_patched = True


def _build():
    _patch_act_tables()
    nc = Bacc(trn_type="TRN2", num_devices=NCORES)
    EIn = dict(kind="ExternalInput")
    # per-core inputs (host-prepped)
    i_xvt = nc.dram_tensor("xvt", [DM, HW], F16, **EIn)
    i_xit = nc.dram_tensor("xit", [DM, HW], F16, **EIn)
    i_wxA = nc.dram_tensor("wxA", [DM, 96], F16, **EIn)  # x in_proj vs xvt
    i_wxB = nc.dram_tensor("wxB", [DM, 96], F16, **EIn)  # x in_proj vs xit
    i_wzA = nc.dram_tensor("wzA", [DM, 48], F16, **EIn)  # z in_proj vs xvt
    i_wzB = nc.dram_tensor("wzB", [DM, 48], F16, **EIn)  # z in_proj vs xit
    i_w9p = nc.dram_tensor("w9p", [96, 9, 96], F16, **EIn)  # conv diag, mods packed
    i_b72p = nc.dram_tensor("b72p", [96, 1], F32, **EIn)
    i_wpk84 = nc.dram_tensor("wpk84", [96, GR], F32, **EIn)  # x_dbl block lhsT
    i_wdtr = nc.dram_tensor("wdtr", [RNK, K, LANES], F16, **EIn)
    i_dtb = nc.dram_tensor("dtb", [LANES, K], F32, **EIn)
    i_asc = nc.dram_tensor("asc", [LANES, K], F32, **EIn)
    i_rep24 = nc.dram_tensor("rep24", [DSL, LANES], F16, **EIn)
    i_repb = nc.dram_tensor("repb", [NST, LANES], F16, **EIn)
    i_m96 = nc.dram_tensor("m96", [LANES, DSL], F16, **EIn)
    i_dvec = nc.dram_tensor("dvec", [DSL, 2], F32, **EIn)  # (vi,ir) summed D
    i_f1q = nc.dram_tensor("f1q", [48, 2, 12], F32, **EIn)  # attn mlp1 per mod
    i_f2 = nc.dram_tensor("f2", [12, 2, 2, DM], F32, **EIn)  # (mod, chunk, out96)
    i_lnw = nc.dram_tensor("lnw", [DM, 2, 4], F32, **EIn)    # (chunk, gvi bvi gir bir)
    i_wout = nc.dram_tensor("wout", [DM, 2, DM], F32, **EIn)  # (contract chunk, out)
    i_wz = nc.dram_tensor("wz", [DM, 4, DM], F16, **EIn)     # z lhsT (vi0,vi1,ir0,ir1)
    i_onec = nc.dram_tensor("onec", [DM, 1], F16, **EIn)
    i_oner = nc.dram_tensor("oner", [1, DM], F32, **EIn)
    i_xvc = nc.dram_tensor("xvc", [DM, PC], F16, **EIn)
    i_xic = nc.dram_tensor("xic", [DM, PC], F16, **EIn)
    o_out = nc.dram_tensor("out", [DM, PC], F32, kind="ExternalOutput")
    # collective DRAM buffers (one pair per r1 col-slice)
    d_r1i = [nc.dram_tensor(f"d_r1i{j}", [GR, SL_COLS[j]], F16)
             for j in range(3)]
    d_r1o = [nc.dram_tensor(f"d_r1o{j}", [GR, SL_COLS[j]], F16,
                            addr_space="Shared") for j in range(3)]
    d_warm = nc.dram_tensor("d_warm", [1, 32], F32)
    d_warmo = nc.dram_tensor("d_warmo", [1, 32], F32, addr_space="Shared")
    d_a2iv = nc.dram_tensor("d_a2iv", [NCORES, DSL, PC], F16)
    d_a2ov = nc.dram_tensor("d_a2ov", [NCORES, DSL, PC], F16)
    d_a2ir = nc.dram_tensor("d_a2ir", [NCORES, DSL, PC], F16)
    d_a2or = nc.dram_tensor("d_a2or", [NCORES, DSL, PC], F16)

    import contextlib
    with TileContext(nc) as tc, contextlib.ExitStack() as ctx:
        wpool = ctx.enter_context(tc.tile_pool(name="weights", bufs=1))
        big = ctx.enter_context(tc.tile_pool(name="big", bufs=1))

        # ---- load weights ----
        def wtile(shape, src, dt=F32, eng=None):
            t = wpool.tile(shape, dt, tag=src.name, name="w_" + src.name)
            (eng or nc.sync).dma_start(out=t, in_=src[:].bitcast(dt)
                                       if dt is R32 else src[:])
            return t
        # warm up the collective engine: tiny AllReduce with no deps absorbs
        # the ~40us CC cold-start while phase A computes.
        t_warm = wpool.tile([1, 32], F32, tag="warm")
        nc.vector.memset(t_warm[:], 0.0)
        nc.sync.dma_start(out=d_warm[:], in_=t_warm[:])
        nc.gpsimd.collective_compute("AllReduce", OP.add, RG,
                                     ins=[d_warm[:]], outs=[d_warmo[:]])

        # first: the tensors needed to start in_proj rowchunk 0
        t_wxA = wtile([DM, 96], i_wxA, F16)
        t_wxB = wtile([DM, 96], i_wxB, F16)
        t_wzA = wtile([DM, 48], i_wzA, F16)
        t_wzB = wtile([DM, 48], i_wzB, F16)
        t_xvt = big.tile([DM, HW], F16)
        t_xit = big.tile([DM, HW], F16)
        for (r0, nr) in ROWCHUNKS:
            cs = slice(r0 * W, (r0 + nr) * W)
            nc.sync.dma_start(out=t_xvt[:, cs], in_=i_xvt[:, cs])
            nc.sync.dma_start(out=t_xit[:, cs], in_=i_xit[:, cs])
        t_w9p = wtile([96, 9, 96], i_w9p, F16, nc.gpsimd)
        t_b72p = wtile([96, 1], i_b72p, F32, nc.gpsimd)
        t_wpk84 = wtile([96, GR], i_wpk84, R32, nc.gpsimd)
        t_wdtr = wtile([RNK, K, LANES], i_wdtr, F16, nc.gpsimd)
        t_dtb = wtile([LANES, K], i_dtb, F32, nc.gpsimd)
        t_asc = wtile([LANES, K], i_asc, F32, nc.gpsimd)
        t_rep24 = wtile([DSL, LANES], i_rep24, F16, nc.gpsimd)
        t_repb = wtile([NST, LANES], i_repb, F16, nc.gpsimd)
        t_m96 = wtile([LANES, DSL], i_m96, F16, nc.gpsimd)
        t_dvec = wtile([DSL, 2], i_dvec, F32, nc.gpsimd)
        t_f1q = wtile([48, 2, 12], i_f1q, F32, nc.gpsimd)
        t_f2 = wtile([12, 2, 2, DM], i_f2, F32, nc.gpsimd)
        t_lnw = wtile([DM, 2, 4], i_lnw, F32, nc.gpsimd)
        t_wout = wtile([DM, 2, DM], i_wout, R32, nc.gpsimd)
        t_wz = wtile([DM, 4, DM], i_wz, F16, nc.gpsimd)
        t_onec = wtile([DM, 1], i_onec, F16, nc.gpsimd)
        t_oner = wtile([1, DM], i_oner, R32, nc.gpsimd)
        t_xvc = wtile([DM, PC], i_xvc, F16, nc.gpsimd)
        t_xic = wtile([DM, PC], i_xic, F16, nc.gpsimd)

        # persistent SBUF
        t_xs96 = big.tile([96, HW], R32, tag="xs96")  # rows: sub@0|vi@32|ir@64
        t_xs24 = big.tile([DSL, HW], F16, tag="xs24")  # base-0 f16 copies
        t_xv24 = big.tile([DSL, HW], F16, tag="xv24")
        t_xi24 = big.tile([DSL, HW], F16, tag="xi24")
        t_yvi = big.tile([DSL, HW], F16, tag="yvi")
        t_yir = big.tile([DSL, HW], F16, tag="yir")

        # =========== PHASE A: upstream (d-sharded, mods packed) ===========
        pa1 = ctx.enter_context(tc.tile_pool(name="pa1", bufs=1))
        with tc.tile_pool(name="pain", bufs=2, space="PSUM") as pain, \
             tc.tile_pool(name="painz", bufs=2, space="PSUM") as painz, \
             tc.tile_pool(name="pacv", bufs=2, space="PSUM") as pacv, \
             tc.tile_pool(name="padb", bufs=2, space="PSUM") as padb:
            t_pad = pa1.tile([96, 50, 50], F16, tag="pad")
            nc.vector.memset(t_pad[:], 0.0)

            t_zc = pa1.tile([48, HW], F32, tag="zc")  # silu(z), vi|ir packed
            t_zacc = pa1.tile([48, len(ROWCHUNKS)], F32, tag="zacc")
            # in_proj (+z) packed: x=[sub|vi|ir], z=[zv|zi]
            for ic, (r0, nr) in enumerate(ROWCHUNKS):
                cols = slice(r0 * W, (r0 + nr) * W)
                p_x = pain.tile([96, RCH], F32, tag="pin")
                nc.tensor.matmul(p_x[:, :nr * W], t_wxA[:], t_xvt[:, cols],
                                 start=True, stop=False)
                nc.tensor.matmul(p_x[:, :nr * W], t_wxB[:], t_xit[:, cols],
                                 start=False, stop=True)
                p_z = painz.tile([48, RCH], F32, tag="pz")
                nc.tensor.matmul(p_z[:, :nr * W], t_wzA[:], t_xvt[:, cols],
                                 start=True, stop=False)
                nc.tensor.matmul(p_z[:, :nr * W], t_wzB[:], t_xit[:, cols],
                                 start=False, stop=True)
                nc.scalar.activation(t_zc[:, cols], p_z[:, :nr * W],
                                     AF.Silu, accum_out=t_zacc[:, ic:ic + 1])
                nc.scalar.copy(
                    t_pad[:, 1 + r0:1 + r0 + nr, 1:49],
                    p_x[:, :nr * W].rearrange("p (a b) -> p a b", a=nr))

            # depthwise conv 3x3 (9 block-diag matmuls) + bias + silu -> xs
            for (r0, nr) in ROWCHUNKS:
                p_c = pacv.tile([96, RCH], F32, tag="pconv")
                for tap in range(9):
                    dy, dx = tap // 3, tap % 3
                    nc.tensor.matmul(
                        p_c[:, :nr * W], t_w9p[:, tap, :],
                        t_pad[:, r0 + dy:r0 + dy + nr, dx:dx + 48],
                        start=(tap == 0), stop=(tap == 8))
                cols = slice(r0 * W, (r0 + nr) * W)
                nc.scalar.activation(
                    t_xs96[:, cols], p_c[:, :nr * W],
                    AF.Silu, bias=t_b72p[:], scale=1.0)
                nc.vector.tensor_copy(t_xs24[:, cols], t_xs96[0:24, cols])
                nc.vector.tensor_copy(t_xv24[:, cols], t_xs96[32:56, cols])
                nc.vector.tensor_copy(t_xi24[:, cols], t_xs96[64:88, cols])

            # x_dbl all 6 groups in one matmul per rowchunk -> r1 slices
            for ic, (r0, nr) in enumerate(ROWCHUNKS):
                p_d = padb.tile([GR, RCH], F32, tag="pxdbl")
                nc.tensor.matmul(p_d[:, :nr * W], t_wpk84[:],
                                 t_xs96[:, r0 * W:(r0 + nr) * W],
                                 start=True, stop=True)
                t_xe = pa1.tile([GR, RCH], F16, tag=f"txe{ic}", name=f"txe{ic}")
                nc.scalar.copy(t_xe[:, :nr * W], p_d[:, :nr * W])
                j = (r0 * W) // SLC
                lc = r0 * W - j * SLC
                nc.sync.dma_start(out=d_r1i[j][:, lc:lc + nr * W],
                                  in_=t_xe[:, :nr * W])
                if ic == 1:
                    nc.gpsimd.collective_compute(
                        "AllReduce", OP.add, RG,
                        ins=[d_r1i[0][:]], outs=[d_r1o[0][:]])
                elif ic == 3:
                    nc.gpsimd.collective_compute(
                        "AllReduce", OP.add, RG,
                        ins=[d_r1i[1][:]], outs=[d_r1o[1][:]])
        # chan-attn pooled stats + v1 partials [12, 4] (own PSUM scope)
        with tc.tile_pool(name="pav1", bufs=1, space="PSUM") as pav1:
            t_pool = pa1.tile([48, 2], F32, tag="tpool")  # (avg, max)
            nc.vector.tensor_reduce(t_pool[:, 0:1], t_zacc[:],
                                    axis=mybir.AxisListType.X, op=OP.add)
            nc.vector.tensor_scalar_mul(t_pool[:, 0:1], t_pool[:, 0:1], 1.0 / HW)
            nc.vector.tensor_reduce(t_pool[:, 1:2], t_zc[:],
                                    axis=mybir.AxisListType.X, op=OP.max)
            t_v1 = pa1.tile([12, 4], F16, tag="tv1")  # (via, vim, ira, irm)
            p_v1 = pav1.tile([12, 4], F32, tag="pv1")
            for mi in range(2):
                for st in range(2):
                    nc.tensor.matmul(p_v1[:, 2 * mi + st:2 * mi + st + 1],
                                     t_f1q[:, mi, :], t_pool[:, st:st + 1],
                                     start=True, stop=True)
            nc.scalar.copy(t_v1[:], p_v1[:])
            nc.sync.dma_start(out=d_r1i[2][0:12, 384:388], in_=t_v1[:])
        nc.gpsimd.collective_compute("AllReduce", OP.add, RG,
                                     ins=[d_r1i[2][:]], outs=[d_r1o[2][:]])

        # z recompute at my positions (independent of scan) — emitted here
        # so it fills engine gaps during the r1 AllReduce stall.
        t_z = {}
        zq = ctx.enter_context(tc.tile_pool(name="zq", bufs=1))
        with tc.tile_pool(name="zp", bufs=2, space="PSUM") as zpp:
            for zi, (mod, ck) in enumerate(
                    (("vi", 0), ("vi", 1), ("ir", 0), ("ir", 1))):
                xt = t_xvc if mod == "vi" else t_xic
                p_z = zpp.tile([DM, PC], F32, tag="pz2")
                nc.tensor.matmul(p_z[:], t_wz[:, zi, :], xt[:],
                                 start=True, stop=True)
                t_e = zq.tile([DM, PC], F32, tag=f"ze{zi}", name=f"ze{zi}")
                nc.scalar.activation(t_e[:], p_z[:], AF.Exp, bias=0.0, scale=-1.0)
                nc.vector.tensor_scalar_add(t_e[:], t_e[:], 1.0)
                t_r = zq.tile([DM, PC], F32, tag=f"zrr{zi}", name=f"zrr{zi}")
                nc.vector.reciprocal(t_r[:], t_e[:])
                tz = zq.tile([DM, PC], F32, tag=f"z{zi}", name=f"z{zi}")
                nc.vector.tensor_mul(tz[:], p_z[:], t_r[:])
                t_z[(mod, ck)] = tz

        # =========== PHASE B: scan middle ===========
        # full-width per-(tile,half) operand tiles; loads ordered by AR slice
        # so the sync queue drains as each sliced AllReduce completes.
        rfp = ctx.enter_context(tc.tile_pool(name="rfull", bufs=1))
        t_Rf, t_Bf, t_Cf = {}, {}, {}
        for tt in range(3):
            for hh in range(2):
                t_Rf[(tt, hh)] = rfp.tile([RNK, HW], F16, tag=f"Rf{tt}{hh}",
                                          name=f"Rf{tt}{hh}")
                t_Bf[(tt, hh)] = rfp.tile([NST, HW], F16, tag=f"Bf{tt}{hh}",
                                          name=f"Bf{tt}{hh}")
                if hh == 1 or tt == 2:  # y-feeding halves need C
                    t_Cf[(tt, hh)] = rfp.tile([NST, HW], F16, tag=f"Cf{tt}{hh}",
                                              name=f"Cf{tt}{hh}")
        for j in range(3):
            g0 = j * SLC
            span = min(SLC, HW - g0)
            for tt in range(3):
                for hh in range(2):
                    rb = ROWBASE[(tt, hh)]
                    nc.sync.dma_start(out=t_Rf[(tt, hh)][:, g0:g0 + span],
                                      in_=d_r1o[j][rb:rb + RNK, 0:span])
                    nc.sync.dma_start(
                        out=t_Bf[(tt, hh)][:, g0:g0 + span],
                        in_=d_r1o[j][rb + RNK:rb + RNK + NST, 0:span])
                    if (tt, hh) in t_Cf:
                        nc.sync.dma_start(
                            out=t_Cf[(tt, hh)][:, g0:g0 + span],
                            in_=d_r1o[j][rb + RNK + NST:rb + 14, 0:span])
        t_v1o = big.tile([12, 4], F16, tag="v1o")
        nc.sync.dma_start(out=t_v1o, in_=d_r1o[2][0:12, 384:388])

        xs_t = {"sub": t_xs24, "vi": t_xv24, "ir": t_xi24}

        def xs_view(t, col, n):
            half = 1 if col >= HW else 0
            mod = (("sub", "vi"), ("sub", "ir"), ("vi", "ir"))[t][half]
            sc = col - HW * half
            return xs_t[mod][0:DSL, sc:sc + n]

        with tc.tile_pool(name="pb", bufs=3) as pb, \
             tc.tile_pool(name="pbd", bufs=2, space="PSUM") as pbd, \
             tc.tile_pool(name="pbp", bufs=1, space="PSUM") as pbp, \
             tc.tile_pool(name="pby", bufs=2, space="PSUM") as pby:
            for t in range(3):
                chunk_order = range(NCH) if t < 2 else range(NCH - 1, -1, -1)
                carry = None
                for c in chunk_order:
                    c0 = c * CH
                    # segment pieces within chunk: (start, end, k) in tile cols
                    k_lo = t if t < 2 else 2
                    k_hi = t if t < 2 else 3
                    if c0 >= HW:
                        pieces = [(c0, c0 + CH, k_hi)]
                    elif c0 + CH <= HW:
                        pieces = [(c0, c0 + CH, k_lo)]
                    else:
                        pieces = [(c0, HW, k_lo), (HW, c0 + CH, k_hi)]

                    p_dts = pbd.tile([LANES, CH], F32, tag="dts")
                    for (s, e, k) in pieces:
                        half = 1 if s >= HW else 0
                        sp0 = s - HW * half
                        nc.tensor.matmul(p_dts[:, s - c0:e - c0], t_wdtr[:, k, :],
                                         t_Rf[(t, half)][:, sp0:sp0 + e - s],
                                         start=True, stop=True)
                    t_et = pb.tile([LANES, CH], F32, tag="et")
                    for (s, e, k) in pieces:
                        nc.scalar.activation(t_et[:, s - c0:e - c0],
                                             p_dts[:, s - c0:e - c0], AF.Exp,
                                             bias=t_dtb[:, k:k + 1], scale=1.0)
                    t_delta = pb.tile([LANES, CH], F32, tag="delta")
                    nc.scalar.activation(t_delta[:], t_et[:], AF.Ln,
                                         bias=1.0, scale=1.0)
                    t_u = pb.tile([DSL, CH], F16, tag="u")
                    for (s, e, _k) in pieces:
                        nc.gpsimd.tensor_mul(t_u[:, s - c0:e - c0],
                                             t_delta[0:DSL, s - c0:e - c0],
                                             xs_view(t, s, e - s))
                    p_u = pbp.tile([LANES, CH], F32, tag="urep")
                    nc.tensor.matmul(p_u[:], t_rep24[:], t_u[:], start=True, stop=True)
                    p_B = pbp.tile([LANES, CH], F32, tag="brep")
                    for (s, e, k) in pieces:
                        half = 1 if s >= HW else 0
                        sp0 = s - HW * half
                        nc.tensor.matmul(p_B[:, s - c0:e - c0], t_repb[:],
                                         t_Bf[(t, half)][:, sp0:sp0 + e - s],
                                         start=True, stop=True)
                    t_bsb = pb.tile([LANES, CH], F16, tag="bsb")
                    nc.scalar.copy(t_bsb[:], p_B[:])
                    t_b = pb.tile([LANES, CH], F16, tag="b")
                    nc.vector.tensor_mul(t_b[:], p_u[:], t_bsb[:])
                    t_a = pb.tile([LANES, CH], F16, tag="a")
                    for (s, e, k) in pieces:
                        nc.scalar.activation(t_a[:, s - c0:e - c0],
                                             t_delta[:, s - c0:e - c0], AF.Exp,
                                             bias=0.0, scale=t_asc[:, k:k + 1])
                    t_h = pb.tile([LANES, CH], F16, tag="h")
                    if t < 2:
                        init = 0.0 if c == 0 else carry[:, CH - 1:CH]
                        nc.vector.tensor_tensor_scan(t_h[:], t_a[:], t_b[:], init,
                                                     OP.mult, OP.add)
                        carry = t_h
                    else:
                        # reverse scan; pieces processed right-to-left
                        for (s, e, k) in reversed(pieces):
                            sl = slice(s - c0, e - c0)
                            if e == L or e == HW:      # scan-time segment start
                                init = 0.0
                            else:
                                init = carry
                            nc.vector.tensor_tensor_scan(
                                t_h[:, sl][:, ::-1], t_a[:, sl][:, ::-1],
                                t_b[:, sl][:, ::-1], init, OP.mult, OP.add)
                            carry = t_h[:, s - c0:s - c0 + 1]

                    # y: only vi/ir halves feed the output
                    ypieces = [((s if t == 2 else max(s, HW)), e, k)
                               for (s, e, k) in pieces if t == 2 or e > HW]
                    if not ypieces:
                        continue
                    y0 = ypieces[0][0] - c0
                    y1 = ypieces[-1][1] - c0
                    p_C = pbp.tile([LANES, CH], F32, tag="crep")
                    for (s, e, k) in ypieces:
                        half = 1 if s >= HW else 0
                        sp0 = s - HW * half
                        nc.tensor.matmul(p_C[:, s - c0:e - c0], t_repb[:],
                                         t_Cf[(t, half)][:, sp0:sp0 + e - s],
                                         start=True, stop=True)
                    t_hc = pb.tile([LANES, CH], F16, tag="hc")
                    nc.vector.tensor_mul(t_hc[:, y0:y1], t_h[:, y0:y1],
                                         p_C[:, y0:y1])
                    p_y = pby.tile([DSL, CH], F32, tag="y")
                    nc.tensor.matmul(p_y[:, y0:y1], t_m96[:], t_hc[:, y0:y1],
                                     start=True, stop=True)
                    # evacuate/accumulate into y_vi / y_ir; on fwd tiles the
                    # D-skip (combined D_k + D_{k+2}) folds into the same op:
                    # y = xs * D + p_y
                    for (s, e, _k) in ypieces:
                        sl = slice(s - c0, e - c0)
                        if t < 2:
                            yt = t_yvi if t == 0 else t_yir
                            nc.vector.scalar_tensor_tensor(
                                yt[:, s - HW:e - HW],
                                xs_view(t, s, e - s),
                                t_dvec[:, t:t + 1], p_y[:, sl],
                                OP.mult, OP.add)
                        elif s < HW:  # t2 k2 -> vi
                            nc.vector.tensor_add(t_yvi[:, s:e], t_yvi[:, s:e],
                                                 p_y[:, sl])
                        else:         # t2 k3 -> ir
                            nc.vector.tensor_add(t_yir[:, s - HW:e - HW],
                                                 t_yir[:, s - HW:e - HW], p_y[:, sl])

        # =========== A2A: reshard y channels -> positions ===========
        for j in range(NCORES):
            nc.sync.dma_start(out=d_a2ir[j, :, :],
                              in_=t_yir[:, j * PC:(j + 1) * PC])
            nc.sync.dma_start(out=d_a2iv[j, :, :],
                              in_=t_yvi[:, j * PC:(j + 1) * PC])
        nc.gpsimd.collective_compute("AllToAll", OP.bypass, RG,
                                     ins=[d_a2ir[:]], outs=[d_a2or[:]])
        nc.gpsimd.collective_compute("AllToAll", OP.bypass, RG,
                                     ins=[d_a2iv[:]], outs=[d_a2ov[:]])

        # =========== PHASE C: LN + gate + out (position-sharded) ===========
        with tc.tile_pool(name="pcq", bufs=2) as pcq, \
             tc.tile_pool(name="pcp", bufs=1, space="PSUM") as pcp:
            # gather y chunks [96, PC] x (2 chunks, 2 mods)
            t_y = {}
            for mod, d_src in (("vi", d_a2ov), ("ir", d_a2or)):
                for ck in range(2):
                    ty = pcq.tile([DM, PC], F16, tag=f"y{mod}{ck}", name=f"y{mod}{ck}")
                    for jj in range(4):
                        j = ck * 4 + jj
                        nc.sync.dma_start(out=ty[jj * DSL:(jj + 1) * DSL, :],
                                          in_=d_src[j, :, :])
                    t_y[(mod, ck)] = ty
            # chan-attn scales s = 1 + sigmoid(f2 @ (relu(va)+relu(vm)))
            t_vr = pcq.tile([12, 4], F32, tag="vr")
            nc.scalar.activation(t_vr[:], t_v1o[:], AF.Relu)
            t_vw = pcq.tile([12, 2], F32, tag="vw")
            nc.vector.tensor_add(t_vw[:, 0:1], t_vr[:, 0:1], t_vr[:, 1:2])
            nc.vector.tensor_add(t_vw[:, 1:2], t_vr[:, 2:3], t_vr[:, 3:4])
            t_s = {}
            for ck in range(2):
                p_ca = pcp.tile([DM, 2], F32, tag="pca")
                for mod_i in range(2):
                    nc.tensor.matmul(p_ca[:, mod_i:mod_i + 1], t_f2[:, mod_i, ck, :],
                                     t_vw[:, mod_i:mod_i + 1],
                                     start=True, stop=True)
                t_e = pcq.tile([DM, 2], F32, tag="cae")
                nc.scalar.activation(t_e[:], p_ca[:], AF.Exp, bias=0.0, scale=-1.0)
                nc.vector.tensor_scalar_add(t_e[:], t_e[:], 1.0)
                t_r = pcq.tile([DM, 2], F32, tag=f"car{ck}", name=f"car{ck}")
                nc.vector.reciprocal(t_r[:], t_e[:])          # sigmoid
                nc.vector.tensor_scalar_add(t_r[:], t_r[:], 1.0)  # 1 + sigmoid
                t_s[ck] = t_r
            # LN per modality
            t_fin = {}
            for mod in ("vi", "ir"):
                p_s1 = pcp.tile([1, PC], F32, tag="s1")
                p_s2 = pcp.tile([1, PC], F32, tag="s2")
                for ck in range(2):
                    nc.tensor.matmul(p_s1[:], t_onec[:],
                                     t_y[(mod, ck)][:], start=(ck == 0),
                                     stop=(ck == 1))
                for ck in range(2):
                    t_sq = pcq.tile([DM, PC], F16, tag="sq")
                    nc.scalar.activation(t_sq[:], t_y[(mod, ck)][:], AF.Square)
                    nc.tensor.matmul(p_s2[:], t_onec[:],
                                     t_sq[:], start=(ck == 0), stop=(ck == 1))
                t_mu = pcq.tile([1, PC], F32, tag="mu")
                nc.vector.tensor_scalar_mul(t_mu[:], p_s1[:], 1.0 / DI)
                t_musq = pcq.tile([1, PC], F32, tag="musq")
                nc.vector.tensor_mul(t_musq[:], t_mu[:], t_mu[:])
                t_var = pcq.tile([1, PC], F32, tag="var")
                nc.vector.scalar_tensor_tensor(t_var[:], p_s2[:], 1.0 / DI,
                                               t_musq[:], OP.mult, OP.subtract)
                t_eps = pcq.tile([1, 1], F32, tag="eps")
                nc.vector.memset(t_eps[:], 1e-5)
                t_lnv = pcq.tile([1, PC], F32, tag="lnv")
                nc.scalar.activation(t_lnv[:], t_var[:], AF.Ln, bias=t_eps[:], scale=1.0)
                t_rstd = pcq.tile([1, PC], R32, tag="rstd")
                nc.scalar.activation(t_rstd[:], t_lnv[:], AF.Exp, bias=0.0, scale=-0.5)
                t_mur = pcq.tile([1, PC], R32, tag="mur")
                nc.vector.tensor_mul(t_mur[:], t_mu[:], t_rstd[:].bitcast(F32))
                p_q = pcp.tile([DM, PC], F32, tag="pq")
                nc.tensor.matmul(p_q[:], t_oner[:], t_rstd[:], start=True, stop=True)
                p_m = pcp.tile([DM, PC], F32, tag="pm")
                nc.tensor.matmul(p_m[:], t_oner[:], t_mur[:], start=True, stop=True)
                gb = {"vi": (0, 1), "ir": (2, 3)}[mod]
                for ck in range(2):
                    t_t = pcq.tile([DM, PC], F32, tag="lt")
                    nc.vector.tensor_mul(t_t[:], t_y[(mod, ck)][:], p_q[:])
                    t_t2 = pcq.tile([DM, PC], F32, tag="lt2")
                    nc.vector.tensor_sub(t_t2[:], t_t[:], p_m[:])
                    t_yn = pcq.tile([DM, PC], F32, tag="yn")
                    nc.scalar.activation(t_yn[:], t_t2[:], AF.Identity,
                                         bias=t_lnw[:, ck, gb[1]:gb[1] + 1],
                                         scale=t_lnw[:, ck, gb[0]:gb[0] + 1])
                    # gate: fin += yn * z * s
                    t_m1 = pcq.tile([DM, PC], F32, tag="m1")
                    nc.vector.tensor_mul(t_m1[:], t_yn[:], t_z[(mod, ck)][:])
                    if mod == "vi":
                        t_f = pcq.tile([DM, PC], F32, tag=f"fin{ck}", name=f"fin{ck}")
                        nc.vector.tensor_scalar_mul(t_f[:], t_m1[:],
                                                    t_s[ck][:, 0:1])
                        t_fin[ck] = t_f
                    else:
                        nc.vector.scalar_tensor_tensor(t_fin[ck][:], t_m1[:],
                                                       t_s[ck][:, 1:2], t_fin[ck][:],
                                                       OP.mult, OP.add)
            p_o = pcp.tile([DM, PC], F32, tag="po")
            for ck in range(2):
                t_finr = pcq.tile([DM, PC], R32, tag=f"finr{ck}", name=f"finr{ck}")
                nc.vector.tensor_copy(t_finr[:], t_fin[ck][:])
                nc.tensor.matmul(p_o[:], t_wout[:, ck, :], t_finr[:],
                                 start=(ck == 0), stop=(ck == 1))
            t_o = pcq.tile([DM, PC], F32, tag="o")
            nc.scalar.copy(t_o[:], p_o[:])
            nc.sync.dma_start(out=o_out[:], in_=t_o[:])

    nc.finalize()
    return nc


def _prep_inputs(inputs):
    """Host-side prep: slice/transpose weights per core. Returns in_maps."""
    g = {k: np.asarray(v, dtype=np.float32) for k, v in inputs.items()}
    x_vi = g["x_vi"].reshape(HW, DM)
    x_ir = g["x_ir"].reshape(HW, DM)
    xvt = np.ascontiguousarray(x_vi.T)
    xit = np.ascontiguousarray(x_ir.T)
    A = -np.exp(g["A_logs"]).reshape(K, DI, NST)
    Ds = g["Ds"].reshape(K, DI)
    in_maps = []
    for c in range(NCORES):
        S = slice(c * DSL, (c + 1) * DSL)
        m = {}
        m["xvt"] = xvt.astype(np.float16)
        m["xit"] = xit.astype(np.float16)
        # packed in_proj lhsT: x blocks sub@0 vi@32 ir@64, z cols [zv | zi]
        wxA = np.zeros((DM, 96), np.float32)
        wxB = np.zeros((DM, 96), np.float32)
        wzA = np.zeros((DM, 48), np.float32)
        wzB = np.zeros((DM, 48), np.float32)
        wxA[:, 0:24] = g["W_sub"][S].T
        wxA[:, 32:56] = g["W_vi"][S].T
        wxB[:, 0:24] = -g["W_sub"][S].T
        wxB[:, 64:88] = g["W_ir"][S].T
        wzA[:, 0:24] = g["W_vi"][DI:][S].T
        wzB[:, 24:48] = g["W_ir"][DI:][S].T
        m["wxA"], m["wxB"] = wxA.astype(np.float16), wxB.astype(np.float16)
        m["wzA"], m["wzB"] = wzA.astype(np.float16), wzB.astype(np.float16)
        w9p = np.zeros((96, 9, 96), np.float32)
        b72p = np.zeros((96, 1), np.float32)
        for nm in ("sub", "vi", "ir"):
            mo = MODOFF[nm]
            cw = g[f"conv_w_{nm}"][S, 0]      # [DSL, 3, 3]
            for tap in range(9):
                for d in range(DSL):
                    w9p[mo + d, tap, mo + d] = cw[d, tap // 3, tap % 3]
            b72p[mo:mo + DSL, 0] = g[f"conv_b_{nm}"][S]
        m["w9p"], m["b72p"] = w9p.astype(np.float16), b72p
        # x_dbl block lhsT: 6 groups (k0s k0v k1s k1i k2v k3i) x 14 rows
        wpk84 = np.zeros((96, GR), np.float32)
        for gi2, (k, nm) in enumerate(
                ((0, "sub"), (0, "vi"), (1, "sub"), (1, "ir"),
                 (2, "vi"), (3, "ir"))):
            mo = MODOFF[nm]
            wpk84[mo:mo + DSL, gi2 * 14:(gi2 + 1) * 14] = \
                g["x_proj_weight"][k][:, S].T
        m["wpk84"] = wpk84
        wdtr = np.zeros((RNK, K, LANES), np.float32)
        dtb = np.zeros((LANES, K), np.float32)
        asc = np.zeros((LANES, K), np.float32)
        for k in range(K):
            for n in range(NST):
                for d in range(DSL):
                    lane = n * DSL + d
                    wdtr[:, k, lane] = g["dt_projs_weight"][k, c * DSL + d, :]
                    dtb[lane, k] = g["dt_projs_bias"][k, c * DSL + d]
                    asc[lane, k] = A[k, c * DSL + d, n]
        m["wdtr"] = wdtr.astype(np.float16)
        m["dtb"], m["asc"] = dtb, asc
        rep24 = np.zeros((DSL, LANES), np.float32)
        repb = np.zeros((NST, LANES), np.float32)
        m96 = np.zeros((LANES, DSL), np.float32)
        for n in range(NST):
            for d in range(DSL):
                rep24[d, n * DSL + d] = 1
                repb[n, n * DSL + d] = 1
                m96[n * DSL + d, d] = 1
        m["rep24"], m["repb"], m["m96"] = (rep24.astype(np.float16),
            repb.astype(np.float16), m96.astype(np.float16))
        dvec = np.zeros((DSL, 2), np.float32)
        dvec[:, 0] = Ds[0, S] + Ds[2, S]
        dvec[:, 1] = Ds[1, S] + Ds[3, S]
        m["dvec"] = dvec
        f1q = np.zeros((48, 2, 12), np.float32)
        f1q[0:24, 0] = g["ca_vi_f1"][:, S].T
        f1q[24:48, 1] = g["ca_ir_f1"][:, S].T
        m["f1q"] = f1q
        f2 = np.zeros((12, 2, 2, DM), np.float32)
        for ck in range(2):
            f2[:, 0, ck] = g["ca_vi_f2"][ck * DM:(ck + 1) * DM].T
            f2[:, 1, ck] = g["ca_ir_f2"][ck * DM:(ck + 1) * DM].T
        m["f2"] = f2
        lnw = np.zeros((DM, 2, 4), np.float32)
        for ck in range(2):
            cs = slice(ck * DM, (ck + 1) * DM)
            lnw[:, ck, 0] = g["ln_vi_g"][cs]
            lnw[:, ck, 1] = g["ln_vi_b"][cs]
            lnw[:, ck, 2] = g["ln_ir_g"][cs]
            lnw[:, ck, 3] = g["ln_ir_b"][cs]
        m["lnw"] = lnw
        wout = np.zeros((DM, 2, DM), np.float32)
        for ck in range(2):
            wout[:, ck] = g["W_out"][:, ck * DM:(ck + 1) * DM].T
        m["wout"] = wout
        wz = np.zeros((DM, 4, DM), np.float32)
        wz[:, 0] = g["W_vi"][DI:][0:DM].T
        wz[:, 1] = g["W_vi"][DI:][DM:DI].T
        wz[:, 2] = g["W_ir"][DI:][0:DM].T
        wz[:, 3] = g["W_ir"][DI:][DM:DI].T
        m["wz"] = wz.astype(np.float16)
        m["onec"] = np.ones((DM, 1), np.float16)
        m["oner"] = np.ones((1, DM), np.float32)
        m["xvc"] = np.ascontiguousarray(xvt[:, c * PC:(c + 1) * PC]).astype(np.float16)
        m["xic"] = np.ascontiguousarray(xit[:, c * PC:(c + 1) * PC]).astype(np.float16)
        in_maps.append(m)
    return in_maps


def kernel(**inputs):
    if "nc" not in _cache:
        _cache["nc"] = _build()
    nc = _cache["nc"]
    in_maps = _prep_inputs(inputs)
    res = run_bass_kernel_spmd(nc, in_maps, core_ids=list(range(NCORES)))
    out = np.zeros((DM, HW), np.float32)
    for c in range(NCORES):
        out[:, c * PC:(c + 1) * PC] = res.results[c]["out"]
    return out.T.reshape(B, H, W, DM).astype(np.float32)
